# revision 20
# baseline (speedup 1.0000x reference)
"""Trainium2 Bass kernel for CustomMeshRasterizer (pytorch3d-style mesh rasterization).

Strategy (tile-based / sort-middle rasterization):
  - Host (numpy, f64): project vertices to NDC, build per-face affine
    coefficients for perspective-correct barycentrics iw_i(px,py) =
    A_i + B_i*px + C_i*py (signs match screen barys; denom = sum of iw).
  - The 128x128 image is cut into 128 rectangular tiles of 8 rows x 16
    cols (= 128 pixels = one SBUF partition dim). Faces are binned per
    tile by exact triangle-rectangle overlap (SAT); tiles are rank-sorted
    by face count and dealt round-robin to the 8 cores so every core sees
    the same per-slot face-count bound (one SPMD program, per-core data).
  - Device per tile: PE matmul (lhsT = [px;py;1] per partition) computes
    a contiguous [iw0|iw1|iw2|denom] stream for 128 pixels x W faces into
    PSUM, sliced into full 512-column matmuls. Z-test: nearest face =
    argmax of denom among faces with min(iw) >= 0 (1/z ordering is
    monotone in denom). Argmax index via one fused (dcand==dmax)*(2^24-j)
    pass + max-reduce. Winner params are fetched per tile with an
    indirect-DMA gather (overlapped with later tiles); barys / zbuf /
    signed edge distances are recomputed per pixel at [128, n_tiles].
"""

import numpy as np

import concourse.bass as bass
import concourse.bacc as bacc
import concourse.tile as tile
from concourse import mybir
from concourse.bass import IndirectOffsetOnAxis
from concourse.bass_utils import run_bass_kernel_spmd

IMAGE = 128
FOCAL = 2.0
EPS = 1e-8
NCORES = 8
TILER = 8    # tile rows
TILEC = 16   # tile cols  (TILER*TILEC == 128 partitions)
NTILES = (IMAGE // TILER) * (IMAGE // TILEC)   # 128 tiles total
TPC = NTILES // NCORES                         # 16 tile-slots per core
K24 = float(2 << 23)   # 2^24
PTW = 32               # ptab row width (f32 slots)
MARGIN = 1e-3

F32 = mybir.dt.float32
I32 = mybir.dt.int32
AX = mybir.AxisListType.X
OP = mybir.AluOpType


# ---------------------------------------------------------------- host side

def _pixel_coords():
    # match reference f32 op order: 1 - 2*(arange+0.5)/W
    ar = np.arange(IMAGE, dtype=np.float32)
    g = (np.float32(1.0) - np.float32(2.0) * (ar + np.float32(0.5)) / np.float32(IMAGE)).astype(np.float32)
    return g  # same grid for xs (cols) and ys (rows)


def _host_tables(verts, R, T, faces):
    v = verts[0].astype(np.float64) @ R[0].astype(np.float64) + T[0].astype(np.float64)
    z = v[:, 2]
    z_s = np.where(np.abs(z) < EPS, EPS, z)
    x = FOCAL * v[:, 0] / z_s
    y = FOCAL * v[:, 1] / z_s
    zc = np.maximum(z, EPS)

    f = faces.astype(np.int64)
    ax, ay, az = x[f[:, 0]], y[f[:, 0]], zc[f[:, 0]]
    bx, by, bz = x[f[:, 1]], y[f[:, 1]], zc[f[:, 1]]
    cx, cy, cz = x[f[:, 2]], y[f[:, 2]], zc[f[:, 2]]

    area = (bx - ax) * (cy - ay) - (by - ay) * (cx - ax)
    area_s = np.where(np.abs(area) < EPS, EPS, area)
    valid = np.abs(area) > EPS

    def coeffs(px_, py_, qx, qy, zv):
        # edge((px_,py_) -> (qx,qy)) evaluated at pixel, / (area_s * zv)
        Be = -(qy - py_)
        Ce = (qx - px_)
        Ae = (qy - py_) * px_ - (qx - px_) * py_
        s = 1.0 / (area_s * zv)
        B, C, A = Be * s, Ce * s, Ae * s
        B[~valid] = 0.0
        C[~valid] = 0.0
        A[~valid] = -1.0
        return B, C, A

    B0, C0, A0 = coeffs(bx, by, cx, cy, az)
    B1, C1, A1 = coeffs(cx, cy, ax, ay, bz)
    B2, C2, A2 = coeffs(ax, ay, bx, by, cz)
    Bd, Cd, Ad = B0 + B1 + B2, C0 + C1 + C2, A0 + A1 + A2

    iwtab = np.stack([B0, C0, A0, B1, C1, A1, B2, C2, A2, Bd, Cd, Ad]).astype(np.float32)

    # seg-distance params
    abx, aby = bx - ax, by - ay
    bcx, bcy = cx - bx, cy - by
    cax, cay = ax - cx, ay - cy
    iab = 1.0 / np.maximum(abx * abx + aby * aby, EPS)
    ibc = 1.0 / np.maximum(bcx * bcx + bcy * bcy, EPS)
    ica = 1.0 / np.maximum(cax * cax + cay * cay, EPS)

    Fn = f.shape[0]
    ptab = np.zeros((Fn, PTW), dtype=np.float32)
    ptab[:, 0:9] = iwtab[0:9].T
    ptab[:, 9] = np.arange(Fn, dtype=np.float32)          # global face id
    ptab[:, 12] = -ax; ptab[:, 13] = -bx; ptab[:, 14] = -cx
    ptab[:, 15] = -ay; ptab[:, 16] = -by; ptab[:, 17] = -cy
    ptab[:, 18] = abx; ptab[:, 19] = bcx; ptab[:, 20] = cax
    ptab[:, 21] = aby; ptab[:, 22] = bcy; ptab[:, 23] = cay
    ptab[:, 24] = iab; ptab[:, 25] = ibc; ptab[:, 26] = ica

    tri = (ax, ay, bx, by, cx, cy)
    return iwtab, ptab, tri, valid


def _tri_rect_overlap(tri, valid, xlo, xhi, ylo, yhi):
    """Exact triangle-rectangle overlap (separating axis), with margin."""
    ax, ay, bx, by, cx, cy = tri
    fx = np.stack([ax, bx, cx], 1)
    fy = np.stack([ay, by, cy], 1)
    ok = (valid
          & (fx.min(1) - MARGIN <= xhi) & (fx.max(1) + MARGIN >= xlo)
          & (fy.min(1) - MARGIN <= yhi) & (fy.max(1) + MARGIN >= ylo))
    corners = ((xlo, ylo), (xlo, yhi), (xhi, ylo), (xhi, yhi))
    for (px_, py_, qx, qy, ox, oy) in (
            (ax, ay, bx, by, cx, cy), (bx, by, cx, cy, ax, ay), (cx, cy, ax, ay, bx, by)):
        ex, ey = qx - px_, qy - py_
        so = ex * (oy - py_) - ey * (ox - px_)
        s = np.sign(so)
        cs = np.stack([ex * (cyy - py_) - ey * (cxx - px_) for cxx, cyy in corners], 1)
        sep = (s[:, None] * cs < -MARGIN).all(1)
        ok &= ~sep
    return ok


def _prepare(verts, R, T, faces):
    g = _pixel_coords()           # descending NDC coords for rows & cols
    iwtab, ptab, tri, valid = _host_tables(verts, R, T, faces)

    nyb, nxb = IMAGE // TILER, IMAGE // TILEC
    rects = []      # (by, bx)
    lists = []
    for byi in range(nyb):
        rows = g[byi * TILER:(byi + 1) * TILER]
        ylo, yhi = rows.min(), rows.max()
        for bxi in range(nxb):
            cols = g[bxi * TILEC:(bxi + 1) * TILEC]
            xlo, xhi = cols.min(), cols.max()
            sel = np.where(_tri_rect_overlap(tri, valid, xlo, xhi, ylo, yhi))[0]
            rects.append((byi, bxi))
            lists.append(sel.astype(np.int64))

    order = np.argsort([-len(l) for l in lists], kind="stable")
    # slot t, core c -> tile order[8*t + c]
    assign = [[int(order[NCORES * t + c]) for t in range(TPC)] for c in range(NCORES)]

    fmax = []
    for t in range(TPC):
        m = max(len(lists[assign[c][t]]) for c in range(NCORES))
        fmax.append(max(8, (m + 7) // 8 * 8))
    coloff = np.concatenate([[0], np.cumsum(fmax)]).astype(np.int64)
    rtot = int(coloff[-1])
    fmax_max = max(fmax)
    # per-tile compact block: 4W stream cols + 128 lhsT cols
    tboff = np.concatenate([[0], np.cumsum([4 * w + 128 for w in fmax])]).astype(np.int64)
    tabw = int(tboff[-1])

    # pad-face coefficient columns (iw = -1 always, never inside)
    pad_iw = np.zeros((12,), dtype=np.float32)
    pad_iw[2] = pad_iw[5] = pad_iw[8] = -1.0
    pad_iw[11] = 1.0

    # partition p of a tile -> (row_local = p // TILEC, col_local = p % TILEC)
    ploc_r = np.arange(128) // TILEC
    ploc_c = np.arange(128) % TILEC

    in_maps = []
    for c in range(NCORES):
        tabs = np.zeros((3, tabw), dtype=np.float32)
        ptabL = np.zeros((rtot, PTW), dtype=np.float32)
        pxq = np.zeros((128, TPC), dtype=np.float32)
        pyq = np.zeros((128, TPC), dtype=np.float32)
        for t in range(TPC):
            ti = assign[c][t]
            byi, bxi = rects[ti]
            lst = lists[ti]
            n, W = len(lst), fmax[t]
            # chunk-major contiguous stream: per chunk ci: iw0|iw1|iw2|den
            pos = int(tboff[t])
            for c0 in range(0, W, 512):
                cw = min(512, W - c0)
                for q in range(4):
                    for k in range(3):
                        row = tabs[k, pos:pos + cw]
                        coefs = iwtab[3 * q + k]
                        take = lst[c0:min(c0 + cw, n)]
                        row[:len(take)] = coefs[take]
                        row[len(take):] = pad_iw[3 * q + k]
                    pos += cw
            px = g[bxi * TILEC + ploc_c]
            py = g[byi * TILER + ploc_r]
            tabs[0, pos:pos + 128] = px
            tabs[1, pos:pos + 128] = py
            tabs[2, pos:pos + 128] = 1.0
            ptabL[coloff[t]: coloff[t] + n] = ptab[lst]
            pxq[:, t] = px
            pyq[:, t] = py
        rowoff = np.broadcast_to(coloff[:-1].astype(np.float32), (128, TPC)).copy()
        in_maps.append({
            "tabs": tabs,
            "ptabL": ptabL,
            "rowoff": rowoff,
            "px3": np.repeat(pxq[:, :, None], 3, axis=2).copy(),
            "py3": np.repeat(pyq[:, :, None], 3, axis=2).copy(),
        })
    meta = {"fmax": fmax, "fmax_max": fmax_max, "tabw": tabw, "tboff": tboff,
            "rtot": rtot, "coloff": coloff,
            "assign": assign, "rects": rects}
    return in_maps, meta


# ---------------------------------------------------------------- device program

def _build_program(meta):
    fmax = meta["fmax"]
    fmax_max = meta["fmax_max"]
    tabw = meta["tabw"]
    tboff = meta["tboff"]
    rtot = meta["rtot"]
    coloff = meta["coloff"]

    nc = bacc.Bacc("TRN2", target_bir_lowering=False, debug=False)

    d_tabs = nc.dram_tensor("tabs", [3, tabw], F32, kind="ExternalInput").ap()
    d_ptab = nc.dram_tensor("ptabL", [rtot, PTW], F32, kind="ExternalInput").ap()
    d_rowoff = nc.dram_tensor("rowoff", [128, TPC], F32, kind="ExternalInput").ap()
    d_px3 = nc.dram_tensor("px3", [128, TPC, 3], F32, kind="ExternalInput").ap()
    d_py3 = nc.dram_tensor("py3", [128, TPC, 3], F32, kind="ExternalInput").ap()

    d_pf = nc.dram_tensor("pf", [128, TPC], I32, kind="ExternalOutput").ap()
    d_zb = nc.dram_tensor("zb", [128, TPC], F32, kind="ExternalOutput").ap()
    d_bary = nc.dram_tensor("bary", [128, TPC, 3], F32, kind="ExternalOutput").ap()
    d_dist = nc.dram_tensor("dist", [128, TPC], F32, kind="ExternalOutput").ap()

    with tile.TileContext(nc) as tc:
        with (
            tc.tile_pool(name="const", bufs=1) as constp,
            tc.tile_pool(name="psum", bufs=2, space="PSUM") as psump,
            tc.tile_pool(name="work", bufs=3) as work,
            tc.tile_pool(name="band", bufs=2) as band,
            tc.tile_pool(name="small", bufs=1) as small,
        ):
            sb_tabs = constp.tile([128, tabw], F32)
            for t in range(TPC):
                o0, o1 = int(tboff[t]), int(tboff[t + 1])
                prow = 32 * (t % 3)
                nc.sync.dma_start(
                    out=sb_tabs[prow:prow + 3, o0:o1], in_=d_tabs[:, o0:o1])
            sb_rowoff = constp.tile([128, TPC], F32)
            nc.sync.dma_start(out=sb_rowoff[:], in_=d_rowoff[:])
            sb_px3 = constp.tile([128, TPC, 3], F32)
            nc.sync.dma_start(out=sb_px3[:], in_=d_px3[:])
            sb_py3 = constp.tile([128, TPC, 3], F32)
            nc.sync.dma_start(out=sb_py3[:], in_=d_py3[:])

            sb_k24 = constp.tile([128, fmax_max], F32)
            nc.gpsimd.iota(
                sb_k24[:], pattern=[[-1, fmax_max]], base=int(K24),
                channel_multiplier=0, allow_small_or_imprecise_dtypes=True,
            )

            dstat = small.tile([128, TPC, 1], F32)   # denom at winner, per tile
            prm = small.tile([128, TPC, PTW], F32)   # gathered winner params

            for t in range(TPC):
                W = fmax[t]
                prow = 32 * (t % 3)
                lhs0 = int(tboff[t]) + 4 * W
                lhsT = sb_tabs[prow:prow + 3, lhs0:lhs0 + 128]
                dcand = band.tile([128, fmax_max], F32, tag="dcand")
                pos = int(tboff[t])
                for c0 in range(0, W, 512):
                    cw = min(512, W - c0)
                    # pack k quantities per 512-col PSUM bank window
                    kq = max(1, min(4, 512 // cw))
                    nwin = -(-4 // kq)
                    ps = psump.tile([128, 4, 512], F32, tag="ps", name=f"ps_{t}_{c0}")
                    qoff = []
                    for w in range(nwin):
                        qlo, qhi = w * kq, min((w + 1) * kq, 4)
                        ww = (qhi - qlo) * cw
                        nc.tensor.matmul(
                            out=ps[:, w, :ww],
                            lhsT=lhsT,
                            rhs=sb_tabs[prow:prow + 3, pos + qlo * cw: pos + qhi * cw],
                            start=True, stop=True,
                        )
                        for q in range(qlo, qhi):
                            qoff.append((w, (q - qlo) * cw))
                    pos += 4 * cw
                    iw0 = ps[:, qoff[0][0], qoff[0][1]:qoff[0][1] + cw]
                    iw1 = ps[:, qoff[1][0], qoff[1][1]:qoff[1][1] + cw]
                    iw2 = ps[:, qoff[2][0], qoff[2][1]:qoff[2][1] + cw]
                    den = ps[:, qoff[3][0], qoff[3][1]:qoff[3][1] + cw]
                    cp1 = work.tile([128, 512], F32, tag="cp1")
                    nc.vector.tensor_copy(out=cp1[:, :cw], in_=iw1)
                    m01 = work.tile([128, 512], F32, tag="m01")
                    nc.vector.tensor_tensor(
                        out=m01[:, :cw], in0=cp1[:, :cw], in1=iw0, op=OP.min)
                    m2 = work.tile([128, 512], F32, tag="m2")
                    nc.vector.tensor_tensor(
                        out=m2[:, :cw], in0=m01[:, :cw], in1=iw2, op=OP.min)
                    nc.vector.scalar_tensor_tensor(
                        out=dcand[:, c0:c0 + cw], in0=m2[:, :cw], scalar=0.0,
                        in1=den, op0=OP.is_ge, op1=OP.mult)
                nc.vector.tensor_reduce(
                    out=dstat[:, t, :], in_=dcand[:, :W], axis=AX, op=OP.max)
                vsel = band.tile([128, fmax_max], F32, tag="vsel")
                nc.vector.scalar_tensor_tensor(
                    out=vsel[:, :W], in0=dcand[:, :W], scalar=dstat[:, t, :],
                    in1=sb_k24[:, :W], op0=OP.is_equal, op1=OP.mult)
                iv = small.tile([128, 1], F32, name=f"iv_{t}", tag=f"iv{t % 4}")
                nc.vector.tensor_reduce(out=iv[:], in_=vsel[:, :W], axis=AX, op=OP.max)
                # local winner idx -> ptab row, gather overlapped with next tiles
                idxg = small.tile([128, 1], F32, name=f"idxg_{t}", tag=f"ix{t % 4}")
                nc.vector.tensor_scalar(
                    out=idxg[:], in0=iv[:], scalar1=K24, op0=OP.subtract,
                    scalar2=-1.0, op1=OP.mult)
                nc.vector.tensor_scalar(
                    out=idxg[:], in0=idxg[:], scalar1=float(coloff[t]), scalar2=None, op0=OP.add)
                idxi = small.tile([128, 1], I32, name=f"idxi_{t}", tag=f"ii{t % 4}")
                nc.vector.tensor_copy(out=idxi[:], in_=idxg[:])
                nc.gpsimd.indirect_dma_start(
                    out=prm[:, t, :], out_offset=None,
                    in_=d_ptab[:],
                    in_offset=IndirectOffsetOnAxis(ap=idxi[:], axis=0),
                )

            # ---- per-pixel finish (all [128,16]-ish) ----
            hit = small.tile([128, TPC, 1], F32)
            nc.vector.tensor_scalar(out=hit[:], in0=dstat[:], scalar1=0.0, scalar2=None, op0=OP.is_gt)
            denc = small.tile([128, TPC, 1], F32)
            nc.vector.tensor_scalar(out=denc[:], in0=dstat[:], scalar1=float(EPS), scalar2=None, op0=OP.max)
            rz = small.tile([128, TPC, 1], F32)
            nc.vector.reciprocal(out=rz[:], in_=denc[:])

            # barycentrics: iw_i = B_i*px + C_i*py + A_i ; b_i = iw_i * rz
            B3 = prm[:, :, 0:9:3]
            C3 = prm[:, :, 1:9:3]
            A3 = prm[:, :, 2:9:3]
            cpy = work.tile([128, TPC, 3], F32, tag="p2a")
            nc.vector.tensor_tensor(out=cpy[:], in0=C3, in1=sb_py3[:], op=OP.mult)
            iw3 = work.tile([128, TPC, 3], F32, tag="p2b")
            nc.vector.tensor_tensor(out=iw3[:], in0=B3, in1=sb_px3[:], op=OP.mult)
            nc.vector.tensor_tensor(out=iw3[:], in0=iw3[:], in1=cpy[:], op=OP.add)
            nc.vector.tensor_tensor(out=iw3[:], in0=iw3[:], in1=A3, op=OP.add)
            b3 = work.tile([128, TPC, 3], F32, tag="p2c")
            nc.vector.tensor_tensor(out=b3[:], in0=iw3[:], in1=rz[:].to_broadcast([128, TPC, 3]), op=OP.mult)
            hb3 = hit[:].to_broadcast([128, TPC, 3])
            ob = work.tile([128, TPC, 3], F32, tag="p2d")
            nc.vector.scalar_tensor_tensor(
                out=ob[:], in0=b3[:], scalar=1.0, in1=hb3, op0=OP.add, op1=OP.mult)
            bary_sb = small.tile([128, TPC, 3], F32)
            nc.vector.tensor_scalar(out=bary_sb[:], in0=ob[:], scalar1=-1.0, scalar2=None, op0=OP.add)
            nc.sync.dma_start(out=d_bary[:], in_=bary_sb[:])

            # signed squared distance to nearest edge of winner (on gpsimd)
            NX3 = prm[:, :, 12:15]
            NY3 = prm[:, :, 15:18]
            UX3 = prm[:, :, 18:21]
            UY3 = prm[:, :, 21:24]
            IU3 = prm[:, :, 24:27]
            pax = work.tile([128, TPC, 3], F32, tag="p2e")
            nc.gpsimd.tensor_tensor(out=pax[:], in0=NX3, in1=sb_px3[:], op=OP.add)
            pay = work.tile([128, TPC, 3], F32, tag="p2f")
            nc.gpsimd.tensor_tensor(out=pay[:], in0=NY3, in1=sb_py3[:], op=OP.add)
            u1 = work.tile([128, TPC, 3], F32, tag="p2g")
            nc.gpsimd.tensor_tensor(out=u1[:], in0=pax[:], in1=UX3, op=OP.mult)
            u2 = work.tile([128, TPC, 3], F32, tag="p2h")
            nc.gpsimd.tensor_tensor(out=u2[:], in0=pay[:], in1=UY3, op=OP.mult)
            nc.gpsimd.tensor_tensor(out=u1[:], in0=u1[:], in1=u2[:], op=OP.add)
            nc.gpsimd.tensor_tensor(out=u1[:], in0=u1[:], in1=IU3, op=OP.mult)
            nc.gpsimd.tensor_scalar(
                out=u1[:], in0=u1[:], scalar1=0.0, op0=OP.max, scalar2=1.0, op1=OP.min)
            nc.gpsimd.tensor_tensor(out=u2[:], in0=u1[:], in1=UX3, op=OP.mult)
            dx = work.tile([128, TPC, 3], F32, tag="p2i")
            nc.gpsimd.tensor_tensor(out=dx[:], in0=pax[:], in1=u2[:], op=OP.subtract)
            nc.gpsimd.tensor_tensor(out=u2[:], in0=u1[:], in1=UY3, op=OP.mult)
            dy = work.tile([128, TPC, 3], F32, tag="p2j")
            nc.gpsimd.tensor_tensor(out=dy[:], in0=pay[:], in1=u2[:], op=OP.subtract)
            nc.gpsimd.tensor_tensor(out=dx[:], in0=dx[:], in1=dx[:], op=OP.mult)
            nc.gpsimd.tensor_tensor(out=dy[:], in0=dy[:], in1=dy[:], op=OP.mult)
            nc.gpsimd.tensor_tensor(out=dx[:], in0=dx[:], in1=dy[:], op=OP.add)
            d2m = small.tile([128, TPC, 1], F32)
            nc.vector.tensor_reduce(out=d2m[:], in_=dx[:], axis=AX, op=OP.min)
            # dists = hit ? -d2 : -1  == hit*(1-d2) - 1
            nc.vector.tensor_scalar(
                out=d2m[:], in0=d2m[:], scalar1=-1.0, op0=OP.mult, scalar2=1.0, op1=OP.add)
            nc.vector.tensor_tensor(out=d2m[:], in0=d2m[:], in1=hit[:], op=OP.mult)
            dist_sb = small.tile([128, TPC], F32)
            nc.vector.tensor_scalar(out=dist_sb[:], in0=d2m[:, :, 0], scalar1=-1.0, scalar2=None, op0=OP.add)
            nc.sync.dma_start(out=d_dist[:], in_=dist_sb[:])

            # zbuf = hit ? 1/denom : -1
            zo = small.tile([128, TPC, 1], F32)
            nc.vector.scalar_tensor_tensor(
                out=zo[:], in0=rz[:], scalar=1.0, in1=hit[:], op0=OP.add, op1=OP.mult)
            zb_sb = small.tile([128, TPC], F32)
            nc.vector.tensor_scalar(out=zb_sb[:], in0=zo[:, :, 0], scalar1=-1.0, scalar2=None, op0=OP.add)
            nc.sync.dma_start(out=d_zb[:], in_=zb_sb[:])

            # pix2face = hit ? gid : -1
            gidw = prm[:, :, 9:10]
            pfo = small.tile([128, TPC, 1], F32)
            nc.vector.scalar_tensor_tensor(
                out=pfo[:], in0=gidw, scalar=1.0, in1=hit[:], op0=OP.add, op1=OP.mult)
            nc.vector.tensor_scalar(out=pfo[:], in0=pfo[:], scalar1=-1.0, scalar2=None, op0=OP.add)
            pf_sb = small.tile([128, TPC], I32)
            nc.vector.tensor_copy(out=pf_sb[:], in_=pfo[:, :, 0])
            nc.sync.dma_start(out=d_pf[:], in_=pf_sb[:])

    nc.compile()
    return nc


# ---------------------------------------------------------------- entry point

def kernel(verts, R, T, faces, _debug_sim=False):
    verts = np.asarray(verts, dtype=np.float32)
    R = np.asarray(R, dtype=np.float32)
    T = np.asarray(T, dtype=np.float32)
    faces_in = np.asarray(faces)
    assert verts.shape[0] == 1, "B=1 only"

    in_maps, meta = _prepare(verts, R, T, faces_in)
    nc = _build_program(meta)

    if _debug_sim:
        from concourse.bass_interp import MultiCoreSim
        sim = MultiCoreSim(nc, num_cores=NCORES, trace=False)
        for c in range(NCORES):
            for k, v in in_maps[c].items():
                sim.cores[c].tensor(k)[:] = v
        sim.simulate(check_with_hw=False)
        results = [
            {k: np.array(sim.cores[c].tensor(k)) for k in ("pf", "zb", "bary", "dist")}
            for c in range(NCORES)
        ]
    else:
        results = run_bass_kernel_spmd(nc, in_maps, core_ids=list(range(NCORES))).results

    assign, rects = meta["assign"], meta["rects"]
    ploc_r = np.arange(128) // TILEC
    ploc_c = np.arange(128) % TILEC
    pf = np.full((1, IMAGE, IMAGE, 1), -1, dtype=np.int32)
    zb = np.full((1, IMAGE, IMAGE, 1), -1.0, dtype=np.float32)
    ba = np.full((1, IMAGE, IMAGE, 1, 3), -1.0, dtype=np.float32)
    di = np.full((1, IMAGE, IMAGE, 1), -1.0, dtype=np.float32)
    for c in range(NCORES):
        res = results[c]
        for t in range(TPC):
            byi, bxi = rects[assign[c][t]]
            rr = byi * TILER + ploc_r
            cc = bxi * TILEC + ploc_c
            pf[0, rr, cc, 0] = res["pf"][:, t]
            zb[0, rr, cc, 0] = res["zb"][:, t]
            ba[0, rr, cc, 0, :] = res["bary"][:, t, :]
            di[0, rr, cc, 0] = res["dist"][:, t]
    return pf, zb, ba, di


# revision 22
# speedup vs baseline: 1.6334x; 1.6334x over previous
"""Trainium2 Bass kernel for CustomMeshRasterizer (pytorch3d-style mesh rasterization).

Strategy (tile-based / sort-middle rasterization):
  - Host (numpy, f64): project vertices to NDC, build per-face affine
    coefficients for perspective-correct barycentrics iw_i(px,py) =
    A_i + B_i*px + C_i*py (signs match screen barys; denom = sum of iw).
  - The 128x128 image is cut into 128 rectangular tiles of 8 rows x 16
    cols (= 128 pixels = one SBUF partition dim). Faces are binned per
    tile by exact triangle-rectangle overlap (SAT); tiles are rank-sorted
    by face count and dealt round-robin to the 8 cores so every core sees
    the same per-slot face-count bound (one SPMD program, per-core data).
  - Device per tile: PE matmul (lhsT = [px;py;1] per partition) computes
    a contiguous [iw0|iw1|iw2|denom] stream for 128 pixels x W faces into
    PSUM, sliced into full 512-column matmuls. Z-test: nearest face =
    argmax of denom among faces with min(iw) >= 0 (1/z ordering is
    monotone in denom). Argmax index via one fused (dcand==dmax)*(2^24-j)
    pass + max-reduce. Winner params are fetched per tile with an
    indirect-DMA gather (overlapped with later tiles); barys / zbuf /
    signed edge distances are recomputed per pixel at [128, n_tiles].
"""

import numpy as np

import concourse.bass as bass
import concourse.bacc as bacc
import concourse.tile as tile
from concourse import mybir
from concourse.bass import IndirectOffsetOnAxis
from concourse.bass_utils import run_bass_kernel_spmd

IMAGE = 128
FOCAL = 2.0
EPS = 1e-8
NCORES = 8
TILER = 8    # tile rows
TILEC = 16   # tile cols  (TILER*TILEC == 128 partitions)
NTILES = (IMAGE // TILER) * (IMAGE // TILEC)   # 128 tiles total
TPC = NTILES // NCORES                         # 16 tile-slots per core
K24 = float(2 << 23)   # 2^24
PTW = 32               # ptab row width (f32 slots)
MARGIN = 1e-3

F32 = mybir.dt.float32
I32 = mybir.dt.int32
AX = mybir.AxisListType.X
OP = mybir.AluOpType


# ---------------------------------------------------------------- host side

def _pixel_coords():
    # match reference f32 op order: 1 - 2*(arange+0.5)/W
    ar = np.arange(IMAGE, dtype=np.float32)
    g = (np.float32(1.0) - np.float32(2.0) * (ar + np.float32(0.5)) / np.float32(IMAGE)).astype(np.float32)
    return g  # same grid for xs (cols) and ys (rows)


def _host_tables(verts, R, T, faces):
    v = verts[0].astype(np.float64) @ R[0].astype(np.float64) + T[0].astype(np.float64)
    z = v[:, 2]
    z_s = np.where(np.abs(z) < EPS, EPS, z)
    x = FOCAL * v[:, 0] / z_s
    y = FOCAL * v[:, 1] / z_s
    zc = np.maximum(z, EPS)

    f = faces.astype(np.int64)
    ax, ay, az = x[f[:, 0]], y[f[:, 0]], zc[f[:, 0]]
    bx, by, bz = x[f[:, 1]], y[f[:, 1]], zc[f[:, 1]]
    cx, cy, cz = x[f[:, 2]], y[f[:, 2]], zc[f[:, 2]]

    area = (bx - ax) * (cy - ay) - (by - ay) * (cx - ax)
    area_s = np.where(np.abs(area) < EPS, EPS, area)
    valid = np.abs(area) > EPS

    def coeffs(px_, py_, qx, qy, zv):
        # edge((px_,py_) -> (qx,qy)) evaluated at pixel, / (area_s * zv)
        Be = -(qy - py_)
        Ce = (qx - px_)
        Ae = (qy - py_) * px_ - (qx - px_) * py_
        s = 1.0 / (area_s * zv)
        B, C, A = Be * s, Ce * s, Ae * s
        B[~valid] = 0.0
        C[~valid] = 0.0
        A[~valid] = -1.0
        return B, C, A

    B0, C0, A0 = coeffs(bx, by, cx, cy, az)
    B1, C1, A1 = coeffs(cx, cy, ax, ay, bz)
    B2, C2, A2 = coeffs(ax, ay, bx, by, cz)
    Bd, Cd, Ad = B0 + B1 + B2, C0 + C1 + C2, A0 + A1 + A2

    iwtab = np.stack([B0, C0, A0, B1, C1, A1, B2, C2, A2, Bd, Cd, Ad]).astype(np.float32)

    # seg-distance params
    abx, aby = bx - ax, by - ay
    bcx, bcy = cx - bx, cy - by
    cax, cay = ax - cx, ay - cy
    iab = 1.0 / np.maximum(abx * abx + aby * aby, EPS)
    ibc = 1.0 / np.maximum(bcx * bcx + bcy * bcy, EPS)
    ica = 1.0 / np.maximum(cax * cax + cay * cay, EPS)

    Fn = f.shape[0]
    ptab = np.zeros((Fn, PTW), dtype=np.float32)
    ptab[:, 0:9] = iwtab[0:9].T
    ptab[:, 9] = np.arange(Fn, dtype=np.float32)          # global face id
    ptab[:, 12] = -ax; ptab[:, 13] = -bx; ptab[:, 14] = -cx
    ptab[:, 15] = -ay; ptab[:, 16] = -by; ptab[:, 17] = -cy
    ptab[:, 18] = abx; ptab[:, 19] = bcx; ptab[:, 20] = cax
    ptab[:, 21] = aby; ptab[:, 22] = bcy; ptab[:, 23] = cay
    ptab[:, 24] = iab; ptab[:, 25] = ibc; ptab[:, 26] = ica

    tri = (ax, ay, bx, by, cx, cy)
    # f64 coefficient arrays for exact host-side occlusion culling
    cull = (B0, C0, A0, B1, C1, A1, B2, C2, A2, Bd, Cd, Ad)
    return iwtab, ptab, tri, valid, cull


def _tri_rect_overlap(tri, valid, xlo, xhi, ylo, yhi):
    """Exact triangle-rectangle overlap (separating axis), with margin."""
    ax, ay, bx, by, cx, cy = tri
    fx = np.stack([ax, bx, cx], 1)
    fy = np.stack([ay, by, cy], 1)
    ok = (valid
          & (fx.min(1) - MARGIN <= xhi) & (fx.max(1) + MARGIN >= xlo)
          & (fy.min(1) - MARGIN <= yhi) & (fy.max(1) + MARGIN >= ylo))
    corners = ((xlo, ylo), (xlo, yhi), (xhi, ylo), (xhi, yhi))
    for (px_, py_, qx, qy, ox, oy) in (
            (ax, ay, bx, by, cx, cy), (bx, by, cx, cy, ax, ay), (cx, cy, ax, ay, bx, by)):
        ex, ey = qx - px_, qy - py_
        so = ex * (oy - py_) - ey * (ox - px_)
        s = np.sign(so)
        cs = np.stack([ex * (cyy - py_) - ey * (cxx - px_) for cxx, cyy in corners], 1)
        sep = (s[:, None] * cs < -MARGIN).all(1)
        ok &= ~sep
    return ok


def _prepare(verts, R, T, faces):
    g = _pixel_coords()           # descending NDC coords for rows & cols
    iwtab, ptab, tri, valid, cull = _host_tables(verts, R, T, faces)
    (B0, C0, A0, B1, C1, A1, B2, C2, A2, Bd, Cd, Ad) = cull

    nyb, nxb = IMAGE // TILER, IMAGE // TILEC
    rects = []      # (by, bx)
    lists = []
    for byi in range(nyb):
        rows = g[byi * TILER:(byi + 1) * TILER]
        ylo, yhi = rows.min(), rows.max()
        for bxi in range(nxb):
            cols = g[bxi * TILEC:(bxi + 1) * TILEC]
            xlo, xhi = cols.min(), cols.max()
            idx = np.where(_tri_rect_overlap(tri, valid, xlo, xhi, ylo, yhi))[0]
            if len(idx):
                # occlusion cull: drop faces provably behind a face that
                # fully covers the tile (denom = 1/z is affine; extrema at
                # rect corners). Margins dwarf any f32 eval noise.
                corners = ((xlo, ylo), (xlo, yhi), (xhi, ylo), (xhi, yhi))
                dcorn = np.stack(
                    [Ad[idx] + Bd[idx] * cx_ + Cd[idx] * cy_ for cx_, cy_ in corners], 1)
                cover = np.ones(len(idx), bool)
                for (B_, C_, A_) in ((B0, C0, A0), (B1, C1, A1), (B2, C2, A2)):
                    wc = np.stack(
                        [A_[idx] + B_[idx] * cx_ + C_[idx] * cy_ for cx_, cy_ in corners], 1)
                    cover &= (wc > 1e-6).all(1)
                if cover.any():
                    floor_d = dcorn[cover].min(1).max()
                    idx = idx[dcorn.max(1) >= floor_d * (1 - 1e-4)]
            rects.append((byi, bxi))
            lists.append(idx.astype(np.int64))

    order = np.argsort([-len(l) for l in lists], kind="stable")
    # slot t, core c -> tile order[8*t + c]
    assign = [[int(order[NCORES * t + c]) for t in range(TPC)] for c in range(NCORES)]

    fmax = []
    for t in range(TPC):
        m = max(len(lists[assign[c][t]]) for c in range(NCORES))
        fmax.append(max(8, (m + 7) // 8 * 8))
    coloff = np.concatenate([[0], np.cumsum(fmax)]).astype(np.int64)
    rtot = int(coloff[-1])
    fmax_max = max(fmax)
    # per-tile compact block: 4W stream cols + 128 lhsT cols
    tboff = np.concatenate([[0], np.cumsum([4 * w + 128 for w in fmax])]).astype(np.int64)
    tabw = int(tboff[-1])

    # pad-face coefficient columns (iw = -1 always, never inside)
    pad_iw = np.zeros((12,), dtype=np.float32)
    pad_iw[2] = pad_iw[5] = pad_iw[8] = -1.0
    pad_iw[11] = 1.0

    # partition p of a tile -> (row_local = p // TILEC, col_local = p % TILEC)
    ploc_r = np.arange(128) // TILEC
    ploc_c = np.arange(128) % TILEC

    in_maps = []
    for c in range(NCORES):
        tabs = np.zeros((3, tabw), dtype=np.float32)
        ptabL = np.zeros((rtot, PTW), dtype=np.float32)
        pxq = np.zeros((128, TPC), dtype=np.float32)
        pyq = np.zeros((128, TPC), dtype=np.float32)
        for t in range(TPC):
            ti = assign[c][t]
            byi, bxi = rects[ti]
            lst = lists[ti]
            n, W = len(lst), fmax[t]
            # chunk-major contiguous stream: per chunk ci: iw0|iw1|iw2|den
            pos = int(tboff[t])
            for c0 in range(0, W, 512):
                cw = min(512, W - c0)
                for q in range(4):
                    for k in range(3):
                        row = tabs[k, pos:pos + cw]
                        coefs = iwtab[3 * q + k]
                        take = lst[c0:min(c0 + cw, n)]
                        row[:len(take)] = coefs[take]
                        row[len(take):] = pad_iw[3 * q + k]
                    pos += cw
            px = g[bxi * TILEC + ploc_c]
            py = g[byi * TILER + ploc_r]
            tabs[0, pos:pos + 128] = px
            tabs[1, pos:pos + 128] = py
            tabs[2, pos:pos + 128] = 1.0
            ptabL[coloff[t]: coloff[t] + n] = ptab[lst]
            pxq[:, t] = px
            pyq[:, t] = py
        rowoff = np.broadcast_to(coloff[:-1].astype(np.float32), (128, TPC)).copy()
        in_maps.append({
            "tabs": tabs,
            "ptabL": ptabL,
            "rowoff": rowoff,
            "px3": np.repeat(pxq[:, :, None], 3, axis=2).copy(),
            "py3": np.repeat(pyq[:, :, None], 3, axis=2).copy(),
        })
    meta = {"fmax": fmax, "fmax_max": fmax_max, "tabw": tabw, "tboff": tboff,
            "rtot": rtot, "coloff": coloff,
            "assign": assign, "rects": rects}
    return in_maps, meta


# ---------------------------------------------------------------- device program

def _build_program(meta):
    fmax = meta["fmax"]
    fmax_max = meta["fmax_max"]
    tabw = meta["tabw"]
    tboff = meta["tboff"]
    rtot = meta["rtot"]
    coloff = meta["coloff"]

    nc = bacc.Bacc("TRN2", target_bir_lowering=False, debug=False)

    d_tabs = nc.dram_tensor("tabs", [3, tabw], F32, kind="ExternalInput").ap()
    d_ptab = nc.dram_tensor("ptabL", [rtot, PTW], F32, kind="ExternalInput").ap()
    d_rowoff = nc.dram_tensor("rowoff", [128, TPC], F32, kind="ExternalInput").ap()
    d_px3 = nc.dram_tensor("px3", [128, TPC, 3], F32, kind="ExternalInput").ap()
    d_py3 = nc.dram_tensor("py3", [128, TPC, 3], F32, kind="ExternalInput").ap()

    d_pf = nc.dram_tensor("pf", [128, TPC], I32, kind="ExternalOutput").ap()
    d_zb = nc.dram_tensor("zb", [128, TPC], F32, kind="ExternalOutput").ap()
    d_bary = nc.dram_tensor("bary", [128, TPC, 3], F32, kind="ExternalOutput").ap()
    d_dist = nc.dram_tensor("dist", [128, TPC], F32, kind="ExternalOutput").ap()

    with tile.TileContext(nc) as tc:
        with (
            tc.tile_pool(name="const", bufs=1) as constp,
            tc.tile_pool(name="psum", bufs=2, space="PSUM") as psump,
            tc.tile_pool(name="work", bufs=3) as work,
            tc.tile_pool(name="band", bufs=2) as band,
            tc.tile_pool(name="small", bufs=1) as small,
        ):
            sb_tabs = constp.tile([128, tabw], F32)
            for t in range(TPC):
                o0, o1 = int(tboff[t]), int(tboff[t + 1])
                prow = 32 * (t % 3)
                nc.sync.dma_start(
                    out=sb_tabs[prow:prow + 3, o0:o1], in_=d_tabs[:, o0:o1])
            sb_rowoff = constp.tile([128, TPC], F32)
            nc.sync.dma_start(out=sb_rowoff[:], in_=d_rowoff[:])
            sb_px3 = constp.tile([128, TPC, 3], F32)
            nc.sync.dma_start(out=sb_px3[:], in_=d_px3[:])
            sb_py3 = constp.tile([128, TPC, 3], F32)
            nc.sync.dma_start(out=sb_py3[:], in_=d_py3[:])

            sb_k24 = constp.tile([128, fmax_max], F32)
            nc.gpsimd.iota(
                sb_k24[:], pattern=[[-1, fmax_max]], base=int(K24),
                channel_multiplier=0, allow_small_or_imprecise_dtypes=True,
            )

            dstat = small.tile([128, TPC, 1], F32)   # denom at winner, per tile
            prm = small.tile([128, TPC, PTW], F32)   # gathered winner params

            for t in range(TPC):
                W = fmax[t]
                prow = 32 * (t % 3)
                lhs0 = int(tboff[t]) + 4 * W
                lhsT = sb_tabs[prow:prow + 3, lhs0:lhs0 + 128]
                dcand = band.tile([128, fmax_max], F32, tag="dcand")
                pos = int(tboff[t])
                for c0 in range(0, W, 512):
                    cw = min(512, W - c0)
                    # pack k quantities per 512-col PSUM bank window
                    kq = max(1, min(4, 512 // cw))
                    nwin = -(-4 // kq)
                    ps = psump.tile([128, 4, 512], F32, tag="ps", name=f"ps_{t}_{c0}")
                    qoff = []
                    for w in range(nwin):
                        qlo, qhi = w * kq, min((w + 1) * kq, 4)
                        ww = (qhi - qlo) * cw
                        nc.tensor.matmul(
                            out=ps[:, w, :ww],
                            lhsT=lhsT,
                            rhs=sb_tabs[prow:prow + 3, pos + qlo * cw: pos + qhi * cw],
                            start=True, stop=True,
                        )
                        for q in range(qlo, qhi):
                            qoff.append((w, (q - qlo) * cw))
                    pos += 4 * cw
                    iw0 = ps[:, qoff[0][0], qoff[0][1]:qoff[0][1] + cw]
                    iw1 = ps[:, qoff[1][0], qoff[1][1]:qoff[1][1] + cw]
                    iw2 = ps[:, qoff[2][0], qoff[2][1]:qoff[2][1] + cw]
                    den = ps[:, qoff[3][0], qoff[3][1]:qoff[3][1] + cw]
                    cp1 = work.tile([128, 512], F32, tag="cp1")
                    nc.vector.tensor_copy(out=cp1[:, :cw], in_=iw1)
                    m01 = work.tile([128, 512], F32, tag="m01")
                    nc.vector.tensor_tensor(
                        out=m01[:, :cw], in0=cp1[:, :cw], in1=iw0, op=OP.min)
                    m2 = work.tile([128, 512], F32, tag="m2")
                    nc.vector.tensor_tensor(
                        out=m2[:, :cw], in0=m01[:, :cw], in1=iw2, op=OP.min)
                    nc.vector.scalar_tensor_tensor(
                        out=dcand[:, c0:c0 + cw], in0=m2[:, :cw], scalar=0.0,
                        in1=den, op0=OP.is_ge, op1=OP.mult)
                nc.vector.tensor_reduce(
                    out=dstat[:, t, :], in_=dcand[:, :W], axis=AX, op=OP.max)
                vsel = band.tile([128, fmax_max], F32, tag="vsel")
                nc.vector.scalar_tensor_tensor(
                    out=vsel[:, :W], in0=dcand[:, :W], scalar=dstat[:, t, :],
                    in1=sb_k24[:, :W], op0=OP.is_equal, op1=OP.mult)
                iv = small.tile([128, 1], F32, name=f"iv_{t}", tag=f"iv{t % 4}")
                nc.vector.tensor_reduce(out=iv[:], in_=vsel[:, :W], axis=AX, op=OP.max)
                # local winner idx -> ptab row, gather overlapped with next tiles
                idxg = small.tile([128, 1], F32, name=f"idxg_{t}", tag=f"ix{t % 4}")
                nc.vector.tensor_scalar(
                    out=idxg[:], in0=iv[:], scalar1=K24, op0=OP.subtract,
                    scalar2=-1.0, op1=OP.mult)
                nc.vector.tensor_scalar(
                    out=idxg[:], in0=idxg[:], scalar1=float(coloff[t]), scalar2=None, op0=OP.add)
                idxi = small.tile([128, 1], I32, name=f"idxi_{t}", tag=f"ii{t % 4}")
                nc.vector.tensor_copy(out=idxi[:], in_=idxg[:])
                nc.gpsimd.indirect_dma_start(
                    out=prm[:, t, :], out_offset=None,
                    in_=d_ptab[:],
                    in_offset=IndirectOffsetOnAxis(ap=idxi[:], axis=0),
                )

            # ---- per-pixel finish (all [128,16]-ish) ----
            hit = small.tile([128, TPC, 1], F32)
            nc.vector.tensor_scalar(out=hit[:], in0=dstat[:], scalar1=0.0, scalar2=None, op0=OP.is_gt)
            denc = small.tile([128, TPC, 1], F32)
            nc.vector.tensor_scalar(out=denc[:], in0=dstat[:], scalar1=float(EPS), scalar2=None, op0=OP.max)
            rz = small.tile([128, TPC, 1], F32)
            nc.vector.reciprocal(out=rz[:], in_=denc[:])

            # barycentrics: iw_i = B_i*px + C_i*py + A_i ; b_i = iw_i * rz
            B3 = prm[:, :, 0:9:3]
            C3 = prm[:, :, 1:9:3]
            A3 = prm[:, :, 2:9:3]
            cpy = work.tile([128, TPC, 3], F32, tag="p2a")
            nc.vector.tensor_tensor(out=cpy[:], in0=C3, in1=sb_py3[:], op=OP.mult)
            iw3 = work.tile([128, TPC, 3], F32, tag="p2b")
            nc.vector.tensor_tensor(out=iw3[:], in0=B3, in1=sb_px3[:], op=OP.mult)
            nc.vector.tensor_tensor(out=iw3[:], in0=iw3[:], in1=cpy[:], op=OP.add)
            nc.vector.tensor_tensor(out=iw3[:], in0=iw3[:], in1=A3, op=OP.add)
            b3 = work.tile([128, TPC, 3], F32, tag="p2c")
            nc.vector.tensor_tensor(out=b3[:], in0=iw3[:], in1=rz[:].to_broadcast([128, TPC, 3]), op=OP.mult)
            hb3 = hit[:].to_broadcast([128, TPC, 3])
            ob = work.tile([128, TPC, 3], F32, tag="p2d")
            nc.vector.scalar_tensor_tensor(
                out=ob[:], in0=b3[:], scalar=1.0, in1=hb3, op0=OP.add, op1=OP.mult)
            bary_sb = small.tile([128, TPC, 3], F32)
            nc.vector.tensor_scalar(out=bary_sb[:], in0=ob[:], scalar1=-1.0, scalar2=None, op0=OP.add)
            nc.sync.dma_start(out=d_bary[:], in_=bary_sb[:])

            # signed squared distance to nearest edge of winner (on gpsimd)
            NX3 = prm[:, :, 12:15]
            NY3 = prm[:, :, 15:18]
            UX3 = prm[:, :, 18:21]
            UY3 = prm[:, :, 21:24]
            IU3 = prm[:, :, 24:27]
            pax = work.tile([128, TPC, 3], F32, tag="p2e")
            nc.gpsimd.tensor_tensor(out=pax[:], in0=NX3, in1=sb_px3[:], op=OP.add)
            pay = work.tile([128, TPC, 3], F32, tag="p2f")
            nc.gpsimd.tensor_tensor(out=pay[:], in0=NY3, in1=sb_py3[:], op=OP.add)
            u1 = work.tile([128, TPC, 3], F32, tag="p2g")
            nc.gpsimd.tensor_tensor(out=u1[:], in0=pax[:], in1=UX3, op=OP.mult)
            u2 = work.tile([128, TPC, 3], F32, tag="p2h")
            nc.gpsimd.tensor_tensor(out=u2[:], in0=pay[:], in1=UY3, op=OP.mult)
            nc.gpsimd.tensor_tensor(out=u1[:], in0=u1[:], in1=u2[:], op=OP.add)
            nc.gpsimd.tensor_tensor(out=u1[:], in0=u1[:], in1=IU3, op=OP.mult)
            nc.gpsimd.tensor_scalar(
                out=u1[:], in0=u1[:], scalar1=0.0, op0=OP.max, scalar2=1.0, op1=OP.min)
            nc.gpsimd.tensor_tensor(out=u2[:], in0=u1[:], in1=UX3, op=OP.mult)
            dx = work.tile([128, TPC, 3], F32, tag="p2i")
            nc.gpsimd.tensor_tensor(out=dx[:], in0=pax[:], in1=u2[:], op=OP.subtract)
            nc.gpsimd.tensor_tensor(out=u2[:], in0=u1[:], in1=UY3, op=OP.mult)
            dy = work.tile([128, TPC, 3], F32, tag="p2j")
            nc.gpsimd.tensor_tensor(out=dy[:], in0=pay[:], in1=u2[:], op=OP.subtract)
            nc.gpsimd.tensor_tensor(out=dx[:], in0=dx[:], in1=dx[:], op=OP.mult)
            nc.gpsimd.tensor_tensor(out=dy[:], in0=dy[:], in1=dy[:], op=OP.mult)
            nc.gpsimd.tensor_tensor(out=dx[:], in0=dx[:], in1=dy[:], op=OP.add)
            d2m = small.tile([128, TPC, 1], F32)
            nc.vector.tensor_reduce(out=d2m[:], in_=dx[:], axis=AX, op=OP.min)
            # dists = hit ? -d2 : -1  == hit*(1-d2) - 1
            nc.vector.tensor_scalar(
                out=d2m[:], in0=d2m[:], scalar1=-1.0, op0=OP.mult, scalar2=1.0, op1=OP.add)
            nc.vector.tensor_tensor(out=d2m[:], in0=d2m[:], in1=hit[:], op=OP.mult)
            dist_sb = small.tile([128, TPC], F32)
            nc.vector.tensor_scalar(out=dist_sb[:], in0=d2m[:, :, 0], scalar1=-1.0, scalar2=None, op0=OP.add)
            nc.sync.dma_start(out=d_dist[:], in_=dist_sb[:])

            # zbuf = hit ? 1/denom : -1
            zo = small.tile([128, TPC, 1], F32)
            nc.vector.scalar_tensor_tensor(
                out=zo[:], in0=rz[:], scalar=1.0, in1=hit[:], op0=OP.add, op1=OP.mult)
            zb_sb = small.tile([128, TPC], F32)
            nc.vector.tensor_scalar(out=zb_sb[:], in0=zo[:, :, 0], scalar1=-1.0, scalar2=None, op0=OP.add)
            nc.sync.dma_start(out=d_zb[:], in_=zb_sb[:])

            # pix2face = hit ? gid : -1
            gidw = prm[:, :, 9:10]
            pfo = small.tile([128, TPC, 1], F32)
            nc.vector.scalar_tensor_tensor(
                out=pfo[:], in0=gidw, scalar=1.0, in1=hit[:], op0=OP.add, op1=OP.mult)
            nc.vector.tensor_scalar(out=pfo[:], in0=pfo[:], scalar1=-1.0, scalar2=None, op0=OP.add)
            pf_sb = small.tile([128, TPC], I32)
            nc.vector.tensor_copy(out=pf_sb[:], in_=pfo[:, :, 0])
            nc.sync.dma_start(out=d_pf[:], in_=pf_sb[:])

    nc.compile()
    return nc


# ---------------------------------------------------------------- entry point

def kernel(verts, R, T, faces, _debug_sim=False):
    verts = np.asarray(verts, dtype=np.float32)
    R = np.asarray(R, dtype=np.float32)
    T = np.asarray(T, dtype=np.float32)
    faces_in = np.asarray(faces)
    assert verts.shape[0] == 1, "B=1 only"

    in_maps, meta = _prepare(verts, R, T, faces_in)
    nc = _build_program(meta)

    if _debug_sim:
        from concourse.bass_interp import MultiCoreSim
        sim = MultiCoreSim(nc, num_cores=NCORES, trace=False)
        for c in range(NCORES):
            for k, v in in_maps[c].items():
                sim.cores[c].tensor(k)[:] = v
        sim.simulate(check_with_hw=False)
        results = [
            {k: np.array(sim.cores[c].tensor(k)) for k in ("pf", "zb", "bary", "dist")}
            for c in range(NCORES)
        ]
    else:
        results = run_bass_kernel_spmd(nc, in_maps, core_ids=list(range(NCORES))).results

    assign, rects = meta["assign"], meta["rects"]
    ploc_r = np.arange(128) // TILEC
    ploc_c = np.arange(128) % TILEC
    pf = np.full((1, IMAGE, IMAGE, 1), -1, dtype=np.int32)
    zb = np.full((1, IMAGE, IMAGE, 1), -1.0, dtype=np.float32)
    ba = np.full((1, IMAGE, IMAGE, 1, 3), -1.0, dtype=np.float32)
    di = np.full((1, IMAGE, IMAGE, 1), -1.0, dtype=np.float32)
    for c in range(NCORES):
        res = results[c]
        for t in range(TPC):
            byi, bxi = rects[assign[c][t]]
            rr = byi * TILER + ploc_r
            cc = bxi * TILEC + ploc_c
            pf[0, rr, cc, 0] = res["pf"][:, t]
            zb[0, rr, cc, 0] = res["zb"][:, t]
            ba[0, rr, cc, 0, :] = res["bary"][:, t, :]
            di[0, rr, cc, 0] = res["dist"][:, t]
    return pf, zb, ba, di
